# revision 2
# baseline (speedup 1.0000x reference)
"""Trainium2 Bass kernel for nn_CustomCosineEmbeddingLoss.

Computes:  mse(y_pred_logits, y_true) + 0.1 * feat_dist_loss(y_feat)
where feat_dist_loss = sum over 8-row chunks of sum_{i<j} (1 - cos(x_i, x_j)).

Math (per 8-row chunk c, with per-row weights R_i ~= 1/||x_i||):
    sum_{i<j} R_i R_j (x_i . x_j) = 0.5 * ( ||sum_i R_i x_i||^2 - sum_i R_i^2 ||x_i||^2 )
The identity is exact for ANY R_i.  The kernel computes
    Q  = sum_c ||s_c||^2        (s_c = sum_i R_i x_i)
on device; the second term is sum_i (R_i ||x_i||)^2 ~= N exactly (R_i is
computed as 1/sqrt(||x_i||^2) from the same bf16 x used in the matmul),
so the host finishes:  feat = 28*n_chunks - 0.5*(Q - N).

Per 128-row group g (16 chunks), S_g[c, d] = sum_p W_g[p, c] x[p, d] is one
PE matmul with the tiny W (mask * R) as the stationary operand.  Outputs of
8 groups are packed into one [128, 512] PSUM tile (pairs of groups share a
32-partition strip via zero-padded W halves and PSUM accumulation), so a
single ACT Square+accumulate per tile produces Q's partials.

Sharding: data-parallel over rows across 8 cores; tiny per-core partial
tensors are combined on the host.
"""

import sys

import numpy as np

for _p in ("/opt/trn_rl_repo",):
    if _p not in sys.path:
        sys.path.insert(0, _p)

import ml_dtypes

import concourse.bacc as bacc
import concourse.bass as bass
import concourse.bass2jax as _bass2jax
import concourse.mybir as mybir
import concourse.tile as tile
from concourse import bass_utils
from concourse import neff as _neff_mod


# ---------------------------------------------------------------------------
# Patch concourse.bass2jax.rename_neff_tensors_and_patch_header: the stock
# version renames nodes[] / def.json vars but NOT metadata.signatures, and the
# PJRT runtime binds I/O buffers via metadata.signatures — leaving old names
# there makes every input land unbound (kernel reads zeros/garbage).
# ---------------------------------------------------------------------------
def _reset_tarinfo(tarinfo):
    tarinfo.uid = tarinfo.gid = 0
    tarinfo.uname = tarinfo.gname = ""
    tarinfo.mtime = 0
    return tarinfo


def _patched_rename_neff(neff_path, mapping):
    import io
    import tarfile
    import tempfile

    import orjson

    with tempfile.TemporaryDirectory() as repack_dir:
        with open(neff_path, "rb") as neff_f:
            old_neff_header = neff_f.read(1024)
            with tarfile.open(fileobj=neff_f, mode="r") as neff_tar:
                neff_tar.extractall(repack_dir)

        with open(f"{repack_dir}/neff.json") as neff_json_f:
            neff_json = orjson.loads(neff_json_f.read())

        for node in neff_json["nodes"]:
            node["name"] = mapping.get(node["name"], node["name"])
            node["output_names"] = [
                mapping.get(name, name) for name in node["output_names"]
            ]
        sigs = neff_json.get("metadata", {}).get("signatures", {})
        for sig in sigs.values():
            for section in ("inputs", "outputs"):
                if section in sig:
                    sig[section] = {
                        mapping.get(name, name): spec
                        for name, spec in sig[section].items()
                    }
        with open(f"{repack_dir}/neff.json", "w") as neff_json_f:
            neff_json_f.write(orjson.dumps(neff_json).decode())

        with open(f"{repack_dir}/sg00/def.json") as def_json_f:
            def_json = orjson.loads(def_json_f.read())
        def_json["var"] = {
            mapping.get(name, name): items for name, items in def_json["var"].items()
        }
        with open(f"{repack_dir}/sg00/def.json", "w") as def_json_f:
            def_json_f.write(orjson.dumps(def_json).decode())

        neff_buffer = io.BytesIO()
        with tarfile.open(fileobj=neff_buffer, mode="w") as neff_tar:
            neff_tar.add(repack_dir, arcname=".", filter=_reset_tarinfo)

        new_neff_data = neff_buffer.getvalue()
        new_neff_header = _neff_mod.make_deterministic_neff_header(
            old_neff_header=old_neff_header,
            new_neff_data=new_neff_data,
        )

    return new_neff_header + new_neff_data


_bass2jax.rename_neff_tensors_and_patch_header = _patched_rename_neff

# ---- problem shapes (hardcoded per contest rules) ----
N_CORES = 8
N_TOTAL = 131072          # total rows of y_feat / y_pred_logits
D = 512                   # feature dim
C = 64                    # logits dim
CHUNK = 8                 # rows per cosine chunk
ALPHA = 0.1
N_PAIRS = 28              # triu(k=1) pairs per 8x8 chunk

ROWS = N_TOTAL // N_CORES  # 16384 rows per core
P = 128                    # SBUF partitions
G = 8                      # 128-row groups per X tile
XT = ROWS // (P * G)       # 16 X tiles per core
NCH = P // CHUNK           # 16 chunks per 128-row group
MSE_H = 2                  # MSE halves
MSE_F = ROWS * C // P // MSE_H  # 4096 free elems per MSE half tile
NDVE = 4                   # norm groups computed on DVE (rest on ACT)

_VER = "_v5"  # version-suffix for DRAM tensor names: busts stale cached executables
_F32 = mybir.dt.float32
_BF16 = mybir.dt.bfloat16


def _build_kernel():
    nc = bacc.Bacc(
        "TRN2",
        target_bir_lowering=False,
        debug=False,
        enable_asserts=False,
    )
    Alu = mybir.AluOpType
    Act = mybir.ActivationFunctionType

    xf = nc.dram_tensor("xf" + _VER, (ROWS, D), _F32, kind="ExternalInput")
    yp = nc.dram_tensor("yp" + _VER, (ROWS, C), _F32, kind="ExternalInput")
    yt = nc.dram_tensor("yt" + _VER, (ROWS, C), _F32, kind="ExternalInput")
    mask = nc.dram_tensor("mask" + _VER, (P, NCH), _BF16, kind="ExternalInput")
    out_q = nc.dram_tensor("out_q" + _VER, (P, XT), _F32, kind="ExternalOutput")
    out_mse = nc.dram_tensor("out_mse" + _VER, (P, MSE_H), _F32, kind="ExternalOutput")

    with tile.TileContext(nc) as tc:
        from contextlib import ExitStack

        with ExitStack() as ctx:
            singles = ctx.enter_context(tc.tile_pool(name="singles", bufs=1))
            xpool = ctx.enter_context(tc.tile_pool(name="xpool", bufs=3))
            scrpool = ctx.enter_context(tc.tile_pool(name="scr", bufs=2))
            small = ctx.enter_context(tc.tile_pool(name="small", bufs=4))
            msepool = ctx.enter_context(tc.tile_pool(name="mse", bufs=2))
            dpool = ctx.enter_context(tc.tile_pool(name="dpool", bufs=2))
            psq = ctx.enter_context(tc.tile_pool(name="psq", bufs=2, space="PSUM"))

            mask_sb = singles.tile([P, NCH], _BF16)
            nc.sync.dma_start(out=mask_sb, in_=mask[:, :])

            qcols = singles.tile([P, XT], _F32)
            msecols = singles.tile([P, MSE_H], _F32)

            # Persistent zero-padded W storage: [slot, group, 32].  For strip
            # j = g//2: even g lives in cols 0:16 of its [*,g,32] slice, odd g
            # in cols 16:32; the other half stays zero forever (memset once).
            wst = singles.tile([P, 2, G, 32], _BF16)
            nc.vector.memset(wst, 0.0)

            # ---------------- MSE part: sum((yp - yt)^2), bf16 ----------------
            ypv = yp[:, :].rearrange("(p a) c -> p (a c)", p=P)  # [128, 8192]
            ytv = yt[:, :].rearrange("(p a) c -> p (a c)", p=P)
            for h in range(MSE_H):
                pt = msepool.tile([P, MSE_F], _BF16, tag="pt")
                tt = msepool.tile([P, MSE_F], _BF16, tag="tt")
                nc.gpsimd.dma_start(out=pt, in_=ypv[:, h * MSE_F : (h + 1) * MSE_F])
                nc.gpsimd.dma_start(out=tt, in_=ytv[:, h * MSE_F : (h + 1) * MSE_F])
                d = dpool.tile([P, MSE_F], _BF16)
                nc.vector.tensor_sub(d, pt, tt)
                d2 = dpool.tile([P, MSE_F], _BF16, tag="d2")
                nc.vector.scalar_tensor_tensor(
                    out=d2,
                    in0=d,
                    scalar=1.0,
                    in1=d,
                    op0=Alu.mult,
                    op1=Alu.mult,
                    accum_out=msecols[:, h : h + 1],
                )

            # ---------------- cosine part ----------------
            # xf rows: index = ((t*G + g)*P + p);  tile t -> [p, g, d]
            xview = xf[:, :].rearrange("(t g p) d -> t p g d", t=XT, g=G, p=P)
            for t in range(XT):
                xt = xpool.tile([P, G, D], _BF16)
                nc.gpsimd.dma_start(out=xt, in_=xview[t])  # SWDGE casts f32->bf16

                # per-row squared norms of the bf16 values
                nsq = small.tile([P, G], _F32, tag="nsq")
                for g in range(G):
                    if g < NDVE:
                        scrd = scrpool.tile([P, D], _BF16, tag="scrd")
                        nc.vector.scalar_tensor_tensor(
                            out=scrd,
                            in0=xt[:, g, :],
                            scalar=1.0,
                            in1=xt[:, g, :],
                            op0=Alu.mult,
                            op1=Alu.mult,
                            accum_out=nsq[:, g : g + 1],
                        )
                    else:
                        scra = scrpool.tile([P, D], _BF16, tag="scra")
                        nc.scalar.activation(
                            out=scra,
                            in_=xt[:, g, :],
                            func=Act.Square,
                            accum_out=nsq[:, g : g + 1],
                        )
                nn_ = small.tile([P, G], _F32, tag="nn")
                nc.scalar.sqrt(nn_, nsq)
                rr = small.tile([P, G], _F32, tag="rr")
                nc.vector.reciprocal(rr, nn_)

                # W = mask * R into the live halves of the persistent storage
                s = t % 2
                for g in range(G):
                    half = (g % 2) * NCH
                    nc.vector.tensor_scalar_mul(
                        wst[:, s, g, half : half + NCH],
                        mask_sb,
                        rr[:, g : g + 1],
                    )

                # stage 1: strip j <- W_{2j}^T x_{2j} + W_{2j+1}^T x_{2j+1}
                ps = psq.tile([P, D], _F32)
                for j in range(P // 32):
                    nc.tensor.matmul(
                        ps[32 * j : 32 * j + 32, :],
                        wst[:, s, 2 * j, :],
                        xt[:, 2 * j, :],
                        start=True,
                        stop=False,
                        tile_position=(0, 32 * j),
                    )
                    nc.tensor.matmul(
                        ps[32 * j : 32 * j + 32, :],
                        wst[:, s, 2 * j + 1, :],
                        xt[:, 2 * j + 1, :],
                        start=False,
                        stop=True,
                        tile_position=(0, 32 * j),
                    )

                # stage 2: qcols[:, t] = sum_d ps^2  (one ACT op per tile)
                scrq = scrpool.tile([P, D], _BF16, tag="scrq")
                nc.scalar.activation(
                    out=scrq,
                    in_=ps,
                    func=Act.Square,
                    accum_out=qcols[:, t : t + 1],
                )

            nc.sync.dma_start(out=out_q[:, :], in_=qcols)
            nc.sync.dma_start(out=out_mse[:, :], in_=msecols)

    nc.compile()
    return nc


_NC_CACHE = {}


def _get_nc():
    if "nc" not in _NC_CACHE:
        _NC_CACHE["nc"] = _build_kernel()
    return _NC_CACHE["nc"]


def _make_mask():
    m = np.zeros((P, NCH), dtype=ml_dtypes.bfloat16)
    for p in range(P):
        m[p, p // CHUNK] = 1.0
    return m


def _finish(results):
    """Host-side reduction of the per-core partial outputs."""
    q = 0.0
    sumsq = 0.0
    for r in results:
        q += float(r["out_q" + _VER].astype(np.float64).sum())
        sumsq += float(r["out_mse" + _VER].astype(np.float64).sum())
    n_chunks = N_TOTAL // CHUNK
    pair_sim_sum = 0.5 * (q - N_TOTAL)
    feat = N_PAIRS * n_chunks - pair_sim_sum
    mse = sumsq / (N_TOTAL * C)
    return np.array(mse + ALPHA * feat, dtype=np.float32)


def _make_in_maps(y_pred_logits, y_feat, y_true):
    yt2 = np.ascontiguousarray(y_true.reshape(N_TOTAL, C)).astype(
        np.float32, copy=False
    )
    yp2 = np.ascontiguousarray(y_pred_logits).astype(np.float32, copy=False)
    xf2 = np.ascontiguousarray(y_feat).astype(np.float32, copy=False)
    mask = _make_mask()

    in_maps = []
    for c in range(N_CORES):
        sl = slice(c * ROWS, (c + 1) * ROWS)
        in_maps.append(
            {
                "xf" + _VER: np.ascontiguousarray(xf2[sl]),
                "yp" + _VER: np.ascontiguousarray(yp2[sl]),
                "yt" + _VER: np.ascontiguousarray(yt2[sl]),
                "mask" + _VER: mask,
            }
        )
    return in_maps


def _run(y_pred_logits, y_feat, y_true, trace=False):
    nc = _get_nc()
    in_maps = _make_in_maps(y_pred_logits, y_feat, y_true)
    res = bass_utils.run_bass_kernel_spmd(
        nc, in_maps, core_ids=list(range(N_CORES)), trace=trace
    )
    return _finish(res.results), res


def _numpy_fallback(y_pred_logits, y_feat, y_true):
    x = np.asarray(y_feat, dtype=np.float32)
    n = x.shape[0]
    chunks = x.reshape(n // CHUNK, CHUNK, D)
    dot = np.einsum("cid,cjd->cij", chunks, chunks)
    norms = np.sqrt(np.einsum("cii->ci", dot))
    sim = dot / (norms[:, None, :] * norms[:, :, None])
    iu = np.triu_indices(CHUNK, k=1)
    feat = (1.0 - sim[:, iu[0], iu[1]]).sum(dtype=np.float64)
    mse = np.mean(
        (
            np.asarray(y_pred_logits, dtype=np.float32)
            - np.asarray(y_true, dtype=np.float32).reshape(-1, C)
        )
        ** 2,
        dtype=np.float64,
    )
    return np.array(mse + ALPHA * feat, dtype=np.float32)


def kernel(y_pred_logits, y_feat, y_true):
    try:
        out, _ = _run(y_pred_logits, y_feat, y_true, trace=False)
        return out
    except Exception as e:
        print(f"kernel: device path failed ({type(e).__name__}: {e}); "
              "falling back to numpy", file=sys.stderr)
        return _numpy_fallback(y_pred_logits, y_feat, y_true)


# revision 3
# speedup vs baseline: 1.2120x; 1.2120x over previous
"""Trainium2 Bass kernel for nn_CustomCosineEmbeddingLoss.

Computes:  mse(y_pred_logits, y_true) + 0.1 * feat_dist_loss(y_feat)
where feat_dist_loss = sum over 8-row chunks of sum_{i<j} (1 - cos(x_i, x_j)).

Math (per 8-row chunk c, with per-row weights R_i ~= 1/||x_i||):
    sum_{i<j} R_i R_j (x_i . x_j) = 0.5 * ( ||sum_i R_i x_i||^2 - sum_i R_i^2 ||x_i||^2 )
The identity is exact for ANY R_i.  The kernel computes
    Q  = sum_c ||s_c||^2        (s_c = sum_i R_i x_i)
on device; the second term is sum_i (R_i ||x_i||)^2 ~= N exactly (R_i is
computed as 1/sqrt(||x_i||^2) from the same bf16 x used in the matmul),
so the host finishes:  feat = 28*n_chunks - 0.5*(Q - N).

Per 128-row group g (16 chunks), S_g[c, d] = sum_p W_g[p, c] x[p, d] is one
PE matmul with the tiny W (mask * R) as the stationary operand.  Outputs of
8 groups are packed into one [128, 512] PSUM tile (pairs of groups share a
32-partition strip via zero-padded W halves and PSUM accumulation), so a
single ACT Square+accumulate per tile produces Q's partials.

Sharding: data-parallel over rows across 8 cores; tiny per-core partial
tensors are combined on the host.
"""

import sys

import numpy as np

for _p in ("/opt/trn_rl_repo",):
    if _p not in sys.path:
        sys.path.insert(0, _p)

import ml_dtypes

import concourse.bacc as bacc
import concourse.bass as bass
import concourse.bass2jax as _bass2jax
import concourse.mybir as mybir
import concourse.tile as tile
from concourse import bass_utils
from concourse import neff as _neff_mod


# ---------------------------------------------------------------------------
# Patch concourse.bass2jax.rename_neff_tensors_and_patch_header: the stock
# version renames nodes[] / def.json vars but NOT metadata.signatures, and the
# PJRT runtime binds I/O buffers via metadata.signatures — leaving old names
# there makes every input land unbound (kernel reads zeros/garbage).
# ---------------------------------------------------------------------------
def _reset_tarinfo(tarinfo):
    tarinfo.uid = tarinfo.gid = 0
    tarinfo.uname = tarinfo.gname = ""
    tarinfo.mtime = 0
    return tarinfo


def _patched_rename_neff(neff_path, mapping):
    import io
    import tarfile
    import tempfile

    import orjson

    with tempfile.TemporaryDirectory() as repack_dir:
        with open(neff_path, "rb") as neff_f:
            old_neff_header = neff_f.read(1024)
            with tarfile.open(fileobj=neff_f, mode="r") as neff_tar:
                neff_tar.extractall(repack_dir)

        with open(f"{repack_dir}/neff.json") as neff_json_f:
            neff_json = orjson.loads(neff_json_f.read())

        for node in neff_json["nodes"]:
            node["name"] = mapping.get(node["name"], node["name"])
            node["output_names"] = [
                mapping.get(name, name) for name in node["output_names"]
            ]
        sigs = neff_json.get("metadata", {}).get("signatures", {})
        for sig in sigs.values():
            for section in ("inputs", "outputs"):
                if section in sig:
                    sig[section] = {
                        mapping.get(name, name): spec
                        for name, spec in sig[section].items()
                    }
        with open(f"{repack_dir}/neff.json", "w") as neff_json_f:
            neff_json_f.write(orjson.dumps(neff_json).decode())

        with open(f"{repack_dir}/sg00/def.json") as def_json_f:
            def_json = orjson.loads(def_json_f.read())
        def_json["var"] = {
            mapping.get(name, name): items for name, items in def_json["var"].items()
        }
        with open(f"{repack_dir}/sg00/def.json", "w") as def_json_f:
            def_json_f.write(orjson.dumps(def_json).decode())

        neff_buffer = io.BytesIO()
        with tarfile.open(fileobj=neff_buffer, mode="w") as neff_tar:
            neff_tar.add(repack_dir, arcname=".", filter=_reset_tarinfo)

        new_neff_data = neff_buffer.getvalue()
        new_neff_header = _neff_mod.make_deterministic_neff_header(
            old_neff_header=old_neff_header,
            new_neff_data=new_neff_data,
        )

    return new_neff_header + new_neff_data


_bass2jax.rename_neff_tensors_and_patch_header = _patched_rename_neff

# ---- problem shapes (hardcoded per contest rules) ----
N_CORES = 8
N_TOTAL = 131072          # total rows of y_feat / y_pred_logits
D = 512                   # feature dim
C = 64                    # logits dim
CHUNK = 8                 # rows per cosine chunk
ALPHA = 0.1
N_PAIRS = 28              # triu(k=1) pairs per 8x8 chunk

ROWS = N_TOTAL // N_CORES  # 16384 rows per core
P = 128                    # SBUF partitions
G = 8                      # 128-row groups per X tile
XT = ROWS // (P * G)       # 16 X tiles per core
NCH = P // CHUNK           # 16 chunks per 128-row group
MSE_H = 2                  # MSE halves
MSE_F = ROWS * C // P // MSE_H  # 4096 free elems per MSE half tile
NDVE = 4                   # norm groups computed on DVE (rest on ACT)

_VER = "_v6"  # version-suffix for DRAM tensor names: busts stale cached executables
_F32 = mybir.dt.float32
_BF16 = mybir.dt.bfloat16


def _build_kernel():
    nc = bacc.Bacc(
        "TRN2",
        target_bir_lowering=False,
        debug=False,
        enable_asserts=False,
    )
    Alu = mybir.AluOpType
    Act = mybir.ActivationFunctionType

    xf = nc.dram_tensor("xf" + _VER, (ROWS, D), _F32, kind="ExternalInput")
    yp = nc.dram_tensor("yp" + _VER, (ROWS, C), _F32, kind="ExternalInput")
    yt = nc.dram_tensor("yt" + _VER, (ROWS, C), _F32, kind="ExternalInput")
    mask = nc.dram_tensor("mask" + _VER, (P, NCH), _BF16, kind="ExternalInput")
    out_q = nc.dram_tensor("out_q" + _VER, (P, XT), _F32, kind="ExternalOutput")
    out_mse = nc.dram_tensor("out_mse" + _VER, (P, MSE_H), _F32, kind="ExternalOutput")

    with tile.TileContext(nc) as tc:
        from contextlib import ExitStack

        with ExitStack() as ctx:
            singles = ctx.enter_context(tc.tile_pool(name="singles", bufs=1))
            xpool = ctx.enter_context(tc.tile_pool(name="xpool", bufs=5))
            scrpool = ctx.enter_context(tc.tile_pool(name="scr", bufs=4))
            small = ctx.enter_context(tc.tile_pool(name="small", bufs=4))
            msepool = ctx.enter_context(tc.tile_pool(name="mse", bufs=2))
            dpool = ctx.enter_context(tc.tile_pool(name="dpool", bufs=2))
            psq = ctx.enter_context(tc.tile_pool(name="psq", bufs=4, space="PSUM"))

            mask_sb = singles.tile([P, NCH], _BF16)
            nc.sync.dma_start(out=mask_sb, in_=mask[:, :])

            qcols = singles.tile([P, XT], _F32)
            msecols = singles.tile([P, MSE_H], _F32)

            # Persistent zero-padded W storage: [slot, group, 32].  For strip
            # j = g//2: even g lives in cols 0:16 of its [*,g,32] slice, odd g
            # in cols 16:32; the other half stays zero forever (memset once).
            wst = singles.tile([P, 4, G, 32], _BF16)
            nc.vector.memset(wst, 0.0)

            # ---------------- MSE part: sum((yp - yt)^2), bf16 ----------------
            ypv = yp[:, :].rearrange("(p a) c -> p (a c)", p=P)  # [128, 8192]
            ytv = yt[:, :].rearrange("(p a) c -> p (a c)", p=P)
            for h in range(MSE_H):
                pt = msepool.tile([P, MSE_F], _BF16, tag="pt")
                tt = msepool.tile([P, MSE_F], _BF16, tag="tt")
                nc.gpsimd.dma_start(out=pt, in_=ypv[:, h * MSE_F : (h + 1) * MSE_F])
                nc.gpsimd.dma_start(out=tt, in_=ytv[:, h * MSE_F : (h + 1) * MSE_F])
                d = dpool.tile([P, MSE_F], _BF16)
                nc.vector.tensor_sub(d, pt, tt)
                d2 = dpool.tile([P, MSE_F], _BF16, tag="d2")
                nc.vector.scalar_tensor_tensor(
                    out=d2,
                    in0=d,
                    scalar=1.0,
                    in1=d,
                    op0=Alu.mult,
                    op1=Alu.mult,
                    accum_out=msecols[:, h : h + 1],
                )

            # ---------------- cosine part ----------------
            # xf rows: index = ((t*G + g)*P + p);  tile t -> [p, g, d]
            xview = xf[:, :].rearrange("(t g p) d -> t p g d", t=XT, g=G, p=P)
            for t in range(XT):
                xt = xpool.tile([P, G, D], _BF16)
                nc.gpsimd.dma_start(out=xt, in_=xview[t])  # SWDGE casts f32->bf16

                # per-row squared norms of the bf16 values
                nsq = small.tile([P, G], _F32, tag="nsq")
                for g in range(G):
                    if g < NDVE:
                        scrd = scrpool.tile([P, D], _BF16, tag="scrd")
                        nc.vector.scalar_tensor_tensor(
                            out=scrd,
                            in0=xt[:, g, :],
                            scalar=1.0,
                            in1=xt[:, g, :],
                            op0=Alu.mult,
                            op1=Alu.mult,
                            accum_out=nsq[:, g : g + 1],
                        )
                    else:
                        scra = scrpool.tile([P, D], _BF16, tag="scra")
                        nc.scalar.activation(
                            out=scra,
                            in_=xt[:, g, :],
                            func=Act.Square,
                            accum_out=nsq[:, g : g + 1],
                        )
                nn_ = small.tile([P, G], _F32, tag="nn")
                nc.scalar.sqrt(nn_, nsq)
                rr = small.tile([P, G], _F32, tag="rr")
                nc.vector.reciprocal(rr, nn_)

                # W = mask * R into the live halves of the persistent storage
                # (2 strided ops: even groups -> cols 0:16 of their slice, odd
                # groups -> cols 16:32; dead halves stay zero from the memset)
                s = t % 4
                wsv = wst[:, s].rearrange("p (j two) c -> p j two c", two=2)
                rrv = rr.rearrange("p (j two) -> p j two", two=2)
                mb = mask_sb.unsqueeze(1).broadcast_to([P, G // 2, NCH])
                nc.vector.tensor_tensor(
                    out=wsv[:, :, 0, 0:NCH],
                    in0=mb,
                    in1=rrv[:, :, 0].unsqueeze(2).broadcast_to([P, G // 2, NCH]),
                    op=mybir.AluOpType.mult,
                )
                nc.vector.tensor_tensor(
                    out=wsv[:, :, 1, NCH : 2 * NCH],
                    in0=mb,
                    in1=rrv[:, :, 1].unsqueeze(2).broadcast_to([P, G // 2, NCH]),
                    op=mybir.AluOpType.mult,
                )

                # stage 1: strip j <- W_{2j}^T x_{2j} + W_{2j+1}^T x_{2j+1}
                ps = psq.tile([P, D], _F32)
                for j in range(P // 32):
                    nc.tensor.matmul(
                        ps[32 * j : 32 * j + 32, :],
                        wst[:, s, 2 * j, :],
                        xt[:, 2 * j, :],
                        start=True,
                        stop=False,
                        tile_position=(0, 32 * j),
                    )
                    nc.tensor.matmul(
                        ps[32 * j : 32 * j + 32, :],
                        wst[:, s, 2 * j + 1, :],
                        xt[:, 2 * j + 1, :],
                        start=False,
                        stop=True,
                        tile_position=(0, 32 * j),
                    )

                # stage 2: qcols[:, t] = sum_d ps^2  (one ACT op per tile)
                scrq = scrpool.tile([P, D], _BF16, tag="scrq")
                nc.scalar.activation(
                    out=scrq,
                    in_=ps,
                    func=Act.Square,
                    accum_out=qcols[:, t : t + 1],
                )

            nc.sync.dma_start(out=out_q[:, :], in_=qcols)
            nc.sync.dma_start(out=out_mse[:, :], in_=msecols)

    nc.compile()
    return nc


_NC_CACHE = {}


def _get_nc():
    if "nc" not in _NC_CACHE:
        _NC_CACHE["nc"] = _build_kernel()
    return _NC_CACHE["nc"]


def _make_mask():
    m = np.zeros((P, NCH), dtype=ml_dtypes.bfloat16)
    for p in range(P):
        m[p, p // CHUNK] = 1.0
    return m


def _finish(results):
    """Host-side reduction of the per-core partial outputs."""
    q = 0.0
    sumsq = 0.0
    for r in results:
        q += float(r["out_q" + _VER].astype(np.float64).sum())
        sumsq += float(r["out_mse" + _VER].astype(np.float64).sum())
    n_chunks = N_TOTAL // CHUNK
    pair_sim_sum = 0.5 * (q - N_TOTAL)
    feat = N_PAIRS * n_chunks - pair_sim_sum
    mse = sumsq / (N_TOTAL * C)
    return np.array(mse + ALPHA * feat, dtype=np.float32)


def _make_in_maps(y_pred_logits, y_feat, y_true):
    yt2 = np.ascontiguousarray(y_true.reshape(N_TOTAL, C)).astype(
        np.float32, copy=False
    )
    yp2 = np.ascontiguousarray(y_pred_logits).astype(np.float32, copy=False)
    xf2 = np.ascontiguousarray(y_feat).astype(np.float32, copy=False)
    mask = _make_mask()

    in_maps = []
    for c in range(N_CORES):
        sl = slice(c * ROWS, (c + 1) * ROWS)
        in_maps.append(
            {
                "xf" + _VER: np.ascontiguousarray(xf2[sl]),
                "yp" + _VER: np.ascontiguousarray(yp2[sl]),
                "yt" + _VER: np.ascontiguousarray(yt2[sl]),
                "mask" + _VER: mask,
            }
        )
    return in_maps


def _run(y_pred_logits, y_feat, y_true, trace=False):
    nc = _get_nc()
    in_maps = _make_in_maps(y_pred_logits, y_feat, y_true)
    res = bass_utils.run_bass_kernel_spmd(
        nc, in_maps, core_ids=list(range(N_CORES)), trace=trace
    )
    return _finish(res.results), res


def _numpy_fallback(y_pred_logits, y_feat, y_true):
    x = np.asarray(y_feat, dtype=np.float32)
    n = x.shape[0]
    chunks = x.reshape(n // CHUNK, CHUNK, D)
    dot = np.einsum("cid,cjd->cij", chunks, chunks)
    norms = np.sqrt(np.einsum("cii->ci", dot))
    sim = dot / (norms[:, None, :] * norms[:, :, None])
    iu = np.triu_indices(CHUNK, k=1)
    feat = (1.0 - sim[:, iu[0], iu[1]]).sum(dtype=np.float64)
    mse = np.mean(
        (
            np.asarray(y_pred_logits, dtype=np.float32)
            - np.asarray(y_true, dtype=np.float32).reshape(-1, C)
        )
        ** 2,
        dtype=np.float64,
    )
    return np.array(mse + ALPHA * feat, dtype=np.float32)


def kernel(y_pred_logits, y_feat, y_true):
    try:
        out, _ = _run(y_pred_logits, y_feat, y_true, trace=False)
        return out
    except Exception as e:
        print(f"kernel: device path failed ({type(e).__name__}: {e}); "
              "falling back to numpy", file=sys.stderr)
        return _numpy_fallback(y_pred_logits, y_feat, y_true)


# revision 4
# speedup vs baseline: 1.3298x; 1.0972x over previous
"""Trainium2 Bass kernel for nn_CustomCosineEmbeddingLoss.

Computes:  mse(y_pred_logits, y_true) + 0.1 * feat_dist_loss(y_feat)
where feat_dist_loss = sum over 8-row chunks of sum_{i<j} (1 - cos(x_i, x_j)).

Math (per 8-row chunk c, with per-row weights R_i ~= 1/||x_i||):
    sum_{i<j} R_i R_j (x_i . x_j) = 0.5 * ( ||sum_i R_i x_i||^2 - sum_i R_i^2 ||x_i||^2 )
The identity is exact for ANY R_i.  The kernel computes
    Q  = sum_c ||s_c||^2        (s_c = sum_i R_i x_i)
on device; the second term is sum_i (R_i ||x_i||)^2 ~= N exactly (R_i is
computed as 1/sqrt(||x_i||^2) from the same bf16 x used in the matmul),
so the host finishes:  feat = 28*n_chunks - 0.5*(Q - N).

Per 128-row group g (16 chunks), S_g[c, d] = sum_p W_g[p, c] x[p, d] is one
PE matmul with the tiny W (mask * R) as the stationary operand.  Outputs of
8 groups are packed into one [128, 512] PSUM tile (pairs of groups share a
32-partition strip via zero-padded W halves and PSUM accumulation), so a
single ACT Square+accumulate per tile produces Q's partials.

Sharding: data-parallel over rows across 8 cores; tiny per-core partial
tensors are combined on the host.
"""

import sys

import numpy as np

for _p in ("/opt/trn_rl_repo",):
    if _p not in sys.path:
        sys.path.insert(0, _p)

import ml_dtypes

import concourse.bacc as bacc
import concourse.bass as bass
import concourse.bass2jax as _bass2jax
import concourse.mybir as mybir
import concourse.tile as tile
from concourse import bass_utils
from concourse import neff as _neff_mod


# ---------------------------------------------------------------------------
# Patch concourse.bass2jax.rename_neff_tensors_and_patch_header: the stock
# version renames nodes[] / def.json vars but NOT metadata.signatures, and the
# PJRT runtime binds I/O buffers via metadata.signatures — leaving old names
# there makes every input land unbound (kernel reads zeros/garbage).
# ---------------------------------------------------------------------------
def _reset_tarinfo(tarinfo):
    tarinfo.uid = tarinfo.gid = 0
    tarinfo.uname = tarinfo.gname = ""
    tarinfo.mtime = 0
    return tarinfo


def _patched_rename_neff(neff_path, mapping):
    import io
    import tarfile
    import tempfile

    import orjson

    with tempfile.TemporaryDirectory() as repack_dir:
        with open(neff_path, "rb") as neff_f:
            old_neff_header = neff_f.read(1024)
            with tarfile.open(fileobj=neff_f, mode="r") as neff_tar:
                neff_tar.extractall(repack_dir)

        with open(f"{repack_dir}/neff.json") as neff_json_f:
            neff_json = orjson.loads(neff_json_f.read())

        for node in neff_json["nodes"]:
            node["name"] = mapping.get(node["name"], node["name"])
            node["output_names"] = [
                mapping.get(name, name) for name in node["output_names"]
            ]
        sigs = neff_json.get("metadata", {}).get("signatures", {})
        for sig in sigs.values():
            for section in ("inputs", "outputs"):
                if section in sig:
                    sig[section] = {
                        mapping.get(name, name): spec
                        for name, spec in sig[section].items()
                    }
        with open(f"{repack_dir}/neff.json", "w") as neff_json_f:
            neff_json_f.write(orjson.dumps(neff_json).decode())

        with open(f"{repack_dir}/sg00/def.json") as def_json_f:
            def_json = orjson.loads(def_json_f.read())
        def_json["var"] = {
            mapping.get(name, name): items for name, items in def_json["var"].items()
        }
        with open(f"{repack_dir}/sg00/def.json", "w") as def_json_f:
            def_json_f.write(orjson.dumps(def_json).decode())

        neff_buffer = io.BytesIO()
        with tarfile.open(fileobj=neff_buffer, mode="w") as neff_tar:
            neff_tar.add(repack_dir, arcname=".", filter=_reset_tarinfo)

        new_neff_data = neff_buffer.getvalue()
        new_neff_header = _neff_mod.make_deterministic_neff_header(
            old_neff_header=old_neff_header,
            new_neff_data=new_neff_data,
        )

    return new_neff_header + new_neff_data


_bass2jax.rename_neff_tensors_and_patch_header = _patched_rename_neff

# ---- problem shapes (hardcoded per contest rules) ----
N_CORES = 8
N_TOTAL = 131072          # total rows of y_feat / y_pred_logits
D = 512                   # feature dim
C = 64                    # logits dim
CHUNK = 8                 # rows per cosine chunk
ALPHA = 0.1
N_PAIRS = 28              # triu(k=1) pairs per 8x8 chunk

ROWS = N_TOTAL // N_CORES  # 16384 rows per core
P = 128                    # SBUF partitions
G = 8                      # 128-row groups per X tile
XT = ROWS // (P * G)       # 16 X tiles per core
NCH = P // CHUNK           # 16 chunks per 128-row group
MSE_H = 2                  # MSE halves
MSE_F = ROWS * C // P // MSE_H  # 4096 free elems per MSE half tile
NDVE = 4                   # norm groups computed on DVE (rest on ACT)

_VER = "_v7"  # version-suffix for DRAM tensor names: busts stale cached executables
_F32 = mybir.dt.float32
_BF16 = mybir.dt.bfloat16


def _build_kernel():
    nc = bacc.Bacc(
        "TRN2",
        target_bir_lowering=False,
        debug=False,
        enable_asserts=False,
    )
    Alu = mybir.AluOpType
    Act = mybir.ActivationFunctionType

    xf = nc.dram_tensor("xf" + _VER, (ROWS, D), _F32, kind="ExternalInput")
    yp = nc.dram_tensor("yp" + _VER, (ROWS, C), _F32, kind="ExternalInput")
    yt = nc.dram_tensor("yt" + _VER, (ROWS, C), _F32, kind="ExternalInput")
    mask = nc.dram_tensor("mask" + _VER, (P, NCH), _BF16, kind="ExternalInput")
    out_q = nc.dram_tensor("out_q" + _VER, (P, XT), _F32, kind="ExternalOutput")
    out_mse = nc.dram_tensor("out_mse" + _VER, (P, MSE_H), _F32, kind="ExternalOutput")

    with tile.TileContext(nc) as tc:
        from contextlib import ExitStack

        with ExitStack() as ctx:
            singles = ctx.enter_context(tc.tile_pool(name="singles", bufs=1))
            xpool = ctx.enter_context(tc.tile_pool(name="xpool", bufs=5))
            scrpool = ctx.enter_context(tc.tile_pool(name="scr", bufs=4))
            small = ctx.enter_context(tc.tile_pool(name="small", bufs=4))
            msepool = ctx.enter_context(tc.tile_pool(name="mse", bufs=2))
            dpool = ctx.enter_context(tc.tile_pool(name="dpool", bufs=2))
            psq = ctx.enter_context(tc.tile_pool(name="psq", bufs=4, space="PSUM"))

            mask_sb = singles.tile([P, NCH], _BF16)
            nc.sync.dma_start(out=mask_sb, in_=mask[:, :])

            qcols = singles.tile([P, XT], _F32)
            msecols = singles.tile([P, MSE_H], _F32)

            # Persistent zero-padded W storage: [slot, group, 32].  For strip
            # j = g//2: even g lives in cols 0:16 of its [*,g,32] slice, odd g
            # in cols 16:32; the other half stays zero forever (memset once).
            wst = singles.tile([P, 4, G, 32], _BF16)
            nc.vector.memset(wst, 0.0)

            # ---------------- cosine part ----------------
            # xf rows: index = ((t*G + g)*P + p);  tile t -> [p, g, d]
            xview = xf[:, :].rearrange("(t g p) d -> t p g d", t=XT, g=G, p=P)
            for t in range(XT):
                xt = xpool.tile([P, G, D], _BF16)
                nc.gpsimd.dma_start(out=xt, in_=xview[t])  # SWDGE casts f32->bf16

                # per-row squared norms of the bf16 values
                nsq = small.tile([P, G], _F32, tag="nsq")
                for g in range(G):
                    if g < NDVE:
                        scrd = scrpool.tile([P, D], _BF16, tag="scrd")
                        nc.vector.scalar_tensor_tensor(
                            out=scrd,
                            in0=xt[:, g, :],
                            scalar=1.0,
                            in1=xt[:, g, :],
                            op0=Alu.mult,
                            op1=Alu.mult,
                            accum_out=nsq[:, g : g + 1],
                        )
                    else:
                        scra = scrpool.tile([P, D], _BF16, tag="scra")
                        nc.scalar.activation(
                            out=scra,
                            in_=xt[:, g, :],
                            func=Act.Square,
                            accum_out=nsq[:, g : g + 1],
                        )
                nn_ = small.tile([P, G], _F32, tag="nn")
                nc.scalar.sqrt(nn_, nsq)
                rr = small.tile([P, G], _F32, tag="rr")
                nc.vector.reciprocal(rr, nn_)

                # W = mask * R into the live halves of the persistent storage
                # (2 strided ops: even groups -> cols 0:16 of their slice, odd
                # groups -> cols 16:32; dead halves stay zero from the memset)
                s = t % 4
                wsv = wst[:, s].rearrange("p (j two) c -> p j two c", two=2)
                rrv = rr.rearrange("p (j two) -> p j two", two=2)
                mb = mask_sb.unsqueeze(1).broadcast_to([P, G // 2, NCH])
                nc.vector.tensor_tensor(
                    out=wsv[:, :, 0, 0:NCH],
                    in0=mb,
                    in1=rrv[:, :, 0].unsqueeze(2).broadcast_to([P, G // 2, NCH]),
                    op=mybir.AluOpType.mult,
                )
                nc.vector.tensor_tensor(
                    out=wsv[:, :, 1, NCH : 2 * NCH],
                    in0=mb,
                    in1=rrv[:, :, 1].unsqueeze(2).broadcast_to([P, G // 2, NCH]),
                    op=mybir.AluOpType.mult,
                )

                # stage 1: strip j <- W_{2j}^T x_{2j} + W_{2j+1}^T x_{2j+1}
                ps = psq.tile([P, D], _F32)
                for j in range(P // 32):
                    nc.tensor.matmul(
                        ps[32 * j : 32 * j + 32, :],
                        wst[:, s, 2 * j, :],
                        xt[:, 2 * j, :],
                        start=True,
                        stop=False,
                        tile_position=(0, 32 * j),
                    )
                    nc.tensor.matmul(
                        ps[32 * j : 32 * j + 32, :],
                        wst[:, s, 2 * j + 1, :],
                        xt[:, 2 * j + 1, :],
                        start=False,
                        stop=True,
                        tile_position=(0, 32 * j),
                    )

                # stage 2: qcols[:, t] = sum_d ps^2  (one ACT op per tile)
                scrq = scrpool.tile([P, D], _BF16, tag="scrq")
                nc.scalar.activation(
                    out=scrq,
                    in_=ps,
                    func=Act.Square,
                    accum_out=qcols[:, t : t + 1],
                )

            # ---------------- MSE part: sum((yp - yt)^2), bf16 ----------------
            ypv = yp[:, :].rearrange("(p a) c -> p (a c)", p=P)  # [128, 8192]
            ytv = yt[:, :].rearrange("(p a) c -> p (a c)", p=P)
            for h in range(MSE_H):
                pt = msepool.tile([P, MSE_F], _BF16, tag="pt")
                tt = msepool.tile([P, MSE_F], _BF16, tag="tt")
                nc.gpsimd.dma_start(out=pt, in_=ypv[:, h * MSE_F : (h + 1) * MSE_F])
                nc.gpsimd.dma_start(out=tt, in_=ytv[:, h * MSE_F : (h + 1) * MSE_F])
                d = dpool.tile([P, MSE_F], _BF16)
                nc.vector.tensor_sub(d, pt, tt)
                d2 = dpool.tile([P, MSE_F], _BF16, tag="d2")
                nc.vector.scalar_tensor_tensor(
                    out=d2,
                    in0=d,
                    scalar=1.0,
                    in1=d,
                    op0=Alu.mult,
                    op1=Alu.mult,
                    accum_out=msecols[:, h : h + 1],
                )

            nc.sync.dma_start(out=out_q[:, :], in_=qcols)
            nc.sync.dma_start(out=out_mse[:, :], in_=msecols)

    nc.compile()
    return nc


_NC_CACHE = {}


def _get_nc():
    if "nc" not in _NC_CACHE:
        _NC_CACHE["nc"] = _build_kernel()
    return _NC_CACHE["nc"]


def _make_mask():
    m = np.zeros((P, NCH), dtype=ml_dtypes.bfloat16)
    for p in range(P):
        m[p, p // CHUNK] = 1.0
    return m


def _finish(results):
    """Host-side reduction of the per-core partial outputs."""
    q = 0.0
    sumsq = 0.0
    for r in results:
        q += float(r["out_q" + _VER].astype(np.float64).sum())
        sumsq += float(r["out_mse" + _VER].astype(np.float64).sum())
    n_chunks = N_TOTAL // CHUNK
    pair_sim_sum = 0.5 * (q - N_TOTAL)
    feat = N_PAIRS * n_chunks - pair_sim_sum
    mse = sumsq / (N_TOTAL * C)
    return np.array(mse + ALPHA * feat, dtype=np.float32)


def _make_in_maps(y_pred_logits, y_feat, y_true):
    yt2 = np.ascontiguousarray(y_true.reshape(N_TOTAL, C)).astype(
        np.float32, copy=False
    )
    yp2 = np.ascontiguousarray(y_pred_logits).astype(np.float32, copy=False)
    xf2 = np.ascontiguousarray(y_feat).astype(np.float32, copy=False)
    mask = _make_mask()

    in_maps = []
    for c in range(N_CORES):
        sl = slice(c * ROWS, (c + 1) * ROWS)
        in_maps.append(
            {
                "xf" + _VER: np.ascontiguousarray(xf2[sl]),
                "yp" + _VER: np.ascontiguousarray(yp2[sl]),
                "yt" + _VER: np.ascontiguousarray(yt2[sl]),
                "mask" + _VER: mask,
            }
        )
    return in_maps


def _run(y_pred_logits, y_feat, y_true, trace=False):
    nc = _get_nc()
    in_maps = _make_in_maps(y_pred_logits, y_feat, y_true)
    res = bass_utils.run_bass_kernel_spmd(
        nc, in_maps, core_ids=list(range(N_CORES)), trace=trace
    )
    return _finish(res.results), res


def _numpy_fallback(y_pred_logits, y_feat, y_true):
    x = np.asarray(y_feat, dtype=np.float32)
    n = x.shape[0]
    chunks = x.reshape(n // CHUNK, CHUNK, D)
    dot = np.einsum("cid,cjd->cij", chunks, chunks)
    norms = np.sqrt(np.einsum("cii->ci", dot))
    sim = dot / (norms[:, None, :] * norms[:, :, None])
    iu = np.triu_indices(CHUNK, k=1)
    feat = (1.0 - sim[:, iu[0], iu[1]]).sum(dtype=np.float64)
    mse = np.mean(
        (
            np.asarray(y_pred_logits, dtype=np.float32)
            - np.asarray(y_true, dtype=np.float32).reshape(-1, C)
        )
        ** 2,
        dtype=np.float64,
    )
    return np.array(mse + ALPHA * feat, dtype=np.float32)


def kernel(y_pred_logits, y_feat, y_true):
    try:
        out, _ = _run(y_pred_logits, y_feat, y_true, trace=False)
        return out
    except Exception as e:
        print(f"kernel: device path failed ({type(e).__name__}: {e}); "
              "falling back to numpy", file=sys.stderr)
        return _numpy_fallback(y_pred_logits, y_feat, y_true)
